# revision 1
# baseline (speedup 1.0000x reference)
"""GQA attention kernel for Trainium2, 8 NeuronCores.

Problem: B=4, T=2048, C=2048, H=16 q-heads, KVH=4 kv-heads, D=128, RoPE,
causal mask, out-projection with bias.

Sharding: (batch b, q-head-group g) -> core 2*b+g. Each core handles one
batch and 8 q heads (= 2 kv heads). The out-projection partial sums of the
two head-groups of a batch are summed on the host (+ bias).

Per-core dataflow (feature-major "T" layouts):
  xT  [C, T]         activations, C on partitions (16 chunks of 128)
  QT  [128, 8, T]    q projections, partition = d within head (f32r, RoPE'd)
  KT  [128, 2, T]    keys (f32r, RoPE'd)
  V   [128, NKT, 256] values, partition = token within 128-tile (f32r)
  S.T [Tk, Tq]       scores computed transposed; softmax along partitions:
                     exp on ACT, column sums via ones-matmul on PE,
                     normalization deferred to O.T (divide by l).
  OT  [128, 8, T]    attention outputs, partition = d (f32r, normalized)
  out [T, C]         partial out-projection (fp32)

All matmuls are float32r (1 cycle/row for N>=256, ~1.6e-4 rel err).
"""

import math

import numpy as np

B, T, C = 4, 2048, 2048
H, KVH, D = 16, 4, 128
HPC = 8      # q heads per core
KVPC = 2     # kv heads per core
P = 128

_compiled = {}


def _build(seq=T, causal=True):
    import concourse.bacc as bacc
    import concourse.mybir as mybir
    import concourse.tile as tile

    f32 = mybir.dt.float32
    f32r = mybir.dt.float32r
    i32 = mybir.dt.int32
    AF = mybir.ActivationFunctionType
    ALU = mybir.AluOpType

    NKT = seq // P          # Tk tiles of 128
    NG = seq // 512         # Tq groups of 512
    KC = C // P             # 16 contraction chunks
    RPH = HPC // KVPC       # q heads per kv head

    nc = bacc.Bacc(None, target_bir_lowering=False)

    xT = nc.dram_tensor("xT", [C, seq], f32r, kind="ExternalInput")
    wqT = nc.dram_tensor("wqT", [C, HPC * D], f32r, kind="ExternalInput")
    wkT = nc.dram_tensor("wkT", [C, KVPC * D], f32r, kind="ExternalInput")
    wvT = nc.dram_tensor("wvT", [C, KVPC * D], f32r, kind="ExternalInput")
    woT = nc.dram_tensor("woT", [HPC * D, C], f32r, kind="ExternalInput")
    maskT = nc.dram_tensor("maskT", [seq, seq], f32, kind="ExternalInput")
    ifT = nc.dram_tensor("ifT", [D, seq], f32, kind="ExternalInput")
    rmat = nc.dram_tensor("rmat", [D, D], f32r, kind="ExternalInput")
    out = nc.dram_tensor("out", [seq, C], f32, kind="ExternalOutput")

    xT_r = xT.rearrange("(kc p) t -> p kc t", p=P)
    wqT_r = wqT.rearrange("(kc p) m -> p kc m", p=P)
    wkT_r = wkT.rearrange("(kc p) m -> p kc m", p=P)
    wvT_r = wvT.rearrange("(kc p) m -> p kc m", p=P)
    woT_r = woT.rearrange("(h p) c -> p h c", p=P)

    with (
        tile.TileContext(nc) as tc,
        tc.tile_pool(name="persist", bufs=1) as persist,
        tc.tile_pool(name="small", bufs=4) as small,
    ):
        QT = persist.tile([P, HPC, seq], f32r)
        KT = persist.tile([P, KVPC, seq], f32r)
        V = persist.tile([P, NKT, KVPC * D], f32r)
        rm = persist.tile([P, D], f32r)
        nc.sync.dma_start(rm[:], rmat[:])
        ones32 = small.tile([P, 1], f32)
        nc.vector.memset(ones32[:], 1.0)
        ones = persist.tile([P, 1], f32r)
        nc.vector.tensor_copy(ones[:], ones32[:])

        # ======== phase 1: trig tables, projections, RoPE ========
        NH = seq // 1024 if seq >= 1024 else 1
        HL = seq // NH  # half length
        with tc.tile_pool(name="ph1", bufs=1) as ph1:
            sinT = ph1.tile([P, seq], f32)
            cosT = ph1.tile([P, seq], f32)
            wv_sb = ph1.tile([P, KC, KVPC * D], f32r)

            # sin/cos tables via range-reduced LUT sin:
            # f = (theta/2pi + shift) mod 1;  sin(2*pi*f)
            with tc.tile_pool(name="trig", bufs=1) as trig:
                tf_ = trig.tile([P, seq], f32, tag="tf")
                nc.sync.dma_start(tf_[:], ifT[:])
                inv2pi = float(1.0 / (2.0 * math.pi))
                for dst, shift in ((sinT, 0.0), (cosT, 0.25)):
                    ty = trig.tile([P, seq], f32, tag="ty")
                    nc.vector.tensor_scalar_mul(ty[:], tf_[:], inv2pi)
                    if shift:
                        nc.vector.tensor_scalar_add(ty[:], ty[:], shift)
                    ti_ = trig.tile([P, seq], i32, tag="ti")
                    nc.vector.tensor_copy(ti_[:], ty[:])
                    tfr = trig.tile([P, seq], f32, tag="tfr")
                    nc.vector.tensor_copy(tfr[:], ti_[:])
                    nc.vector.tensor_tensor(ty[:], ty[:], tfr[:], ALU.subtract)
                    nc.scalar.activation(dst[:], ty[:], AF.Sin,
                                         scale=float(2.0 * math.pi))

            with (
                tc.tile_pool(name="xh", bufs=2) as xhp,
                tc.tile_pool(name="wm", bufs=3) as wmp,
                tc.tile_pool(name="praw", bufs=1) as praw,
                tc.tile_pool(name="ps1", bufs=2, space="PSUM") as ps1,
            ):
                NQ = seq // 512
                for hf in range(NQ):
                    gch = slice(hf * 512, (hf + 1) * 512)
                    xq = xhp.tile([P, KC, 512], f32r, tag="xq")
                    nc.sync.dma_start(xq[:], xT_r[:, :, gch])
                    if hf == 0:
                        nc.sync.dma_start(wv_sb[:], wvT_r[:])

                    # Q then K projections + RoPE
                    for nm, wr, dstT in ((HPC, wqT_r, QT), (KVPC, wkT_r, KT)):
                        for m in range(nm):
                            wa = wmp.tile([P, KC // 2, P], f32r, tag="wm")
                            nc.sync.dma_start(
                                wa[:], wr[:, :KC // 2, m * P:(m + 1) * P])
                            wb = wmp.tile([P, KC // 2, P], f32r, tag="wm")
                            nc.sync.dma_start(
                                wb[:], wr[:, KC // 2:, m * P:(m + 1) * P])
                            psq = ps1.tile([P, 512], f32, tag="pq")
                            for kc in range(KC):
                                wt = wa if kc < KC // 2 else wb
                                nc.tensor.matmul(
                                    psq[:], wt[:, kc % (KC // 2), :],
                                    xq[:, kc, :],
                                    start=(kc == 0), stop=(kc == KC - 1))
                            qr = praw.tile([P, 512], f32r, tag="qr")
                            nc.scalar.activation(qr[:], psq[:], AF.Copy)
                            psr = ps1.tile([P, 512], f32, tag="pr")
                            nc.tensor.matmul(psr[:], rm[:], qr[:],
                                             start=True, stop=True)
                            # dst = qr*cos + rot*sin (rot*sin in-place in PSUM)
                            nc.vector.tensor_tensor(
                                psr[:], psr[:], sinT[:, gch], ALU.mult)
                            nc.vector.tensor_tensor(
                                dstT[:, m, gch], qr[:].bitcast(f32),
                                cosT[:, gch], ALU.mult)
                            nc.vector.tensor_tensor(
                                dstT[:, m, gch],
                                dstT[:, m, gch].bitcast(f32), psr[:],
                                ALU.add)

                    # V projection
                    for tt in range(4):
                        gtt = hf * 4 + tt
                        psv = ps1.tile([P, KVPC * D], f32, tag="pv")
                        for kc in range(KC):
                            nc.tensor.matmul(
                                psv[:], xq[:, kc, tt * P:(tt + 1) * P],
                                wv_sb[:, kc, :],
                                start=(kc == 0), stop=(kc == KC - 1))
                        nc.scalar.activation(V[:, gtt, :], psv[:], AF.Copy)

        # ======== phases 2+3 ========
        with tc.tile_pool(name="otp", bufs=1) as otp:
            OT = otp.tile([P, HPC, seq], f32r)

            with (
                tc.tile_pool(name="mb", bufs=2) as mbp,
                tc.tile_pool(name="pch", bufs=3) as pch,
                tc.tile_pool(name="lbp", bufs=2) as lbp,
                tc.tile_pool(name="sm2", bufs=4) as sm2,
                tc.tile_pool(name="ps_s", bufs=4, space="PSUM") as ps_s,
                tc.tile_pool(name="ps_o", bufs=2, space="PSUM") as ps_o,
                tc.tile_pool(name="ps_l", bufs=2, space="PSUM") as ps_l,
            ):
                for g in range(NG):
                    qsl = slice(g * 512, (g + 1) * 512)
                    nb = 4 if causal else NKT
                    i0 = 4 * g if causal else 0
                    ntk = 4 * (g + 1) if causal else NKT
                    mb = mbp.tile([P, nb, 512], f32, tag="mb")
                    nc.sync.dma_start(
                        mb[:],
                        maskT[i0 * P:(i0 + nb) * P, qsl].rearrange(
                            "(i p) t -> p i t", p=P))
                    for h in range(HPC):
                        kv = h // RPH
                        pso = ps_o.tile([P, 512], f32, tag="o")
                        psl = ps_l.tile([1, 512], f32, tag="l")

                        # software pipeline: keep 2 S-matmuls in flight ahead
                        # of the exp-dependent l/O matmuls so the PE never
                        # stalls on the DVE-mask -> ACT-exp chain.
                        def emit_s(j):
                            pss = ps_s.tile([P, 512], f32, tag="s")
                            nc.tensor.matmul(
                                pss[:], KT[:, kv, j * P:(j + 1) * P],
                                QT[:, h, qsl], start=True, stop=True)
                            if j >= i0:
                                nc.vector.tensor_tensor(
                                    pss[:], pss[:], mb[:, j - i0, :], ALU.add)
                            return pss

                        sq = [emit_s(j) for j in range(min(2, ntk))]
                        for i in range(ntk):
                            pss = sq.pop(0)
                            pc = pch.tile([P, 512], f32r, tag="p")
                            nc.scalar.activation(pc[:], pss[:], AF.Exp)
                            if i + 2 < ntk:
                                sq.append(emit_s(i + 2))
                            nc.tensor.matmul(
                                psl[:], ones[:], pc[:],
                                start=(i == 0), stop=(i == ntk - 1))
                            nc.tensor.matmul(
                                pso[:], V[:, i, kv * D:(kv + 1) * D], pc[:],
                                start=(i == 0), stop=(i == ntk - 1))
                        lsb = sm2.tile([1, 512], f32, tag="lsb")
                        nc.vector.tensor_copy(lsb[:], psl[:])
                        lrc = sm2.tile([1, 512], f32, tag="lrc")
                        nc.vector.reciprocal(lrc[:], lsb[:])
                        lb = lbp.tile([P, 512], f32, tag="lb")
                        nc.gpsimd.partition_broadcast(lb[:], lrc[:])
                        nc.vector.tensor_tensor(
                            OT[:, h, qsl], pso[:], lb[:], ALU.mult)

            # out-projection, co-chunk outer so weight slices stream once
            with (
                tc.tile_pool(name="wo", bufs=12) as wop,
                tc.tile_pool(name="ob", bufs=3) as obp,
                tc.tile_pool(name="ps3", bufs=4, space="PSUM") as ps3,
            ):
                for co in range(C // 512):
                    csl = slice(co * 512, (co + 1) * 512)
                    woh = []
                    for h in range(HPC):
                        w = wop.tile([P, 512], f32r, tag="wo")
                        nc.sync.dma_start(w[:], woT_r[:, h, csl])
                        woh.append(w)
                    for tt in range(NKT):
                        pso3 = ps3.tile([P, 512], f32, tag="po")
                        for h in range(HPC):
                            nc.tensor.matmul(
                                pso3[:], OT[:, h, tt * P:(tt + 1) * P],
                                woh[h][:],
                                start=(h == 0), stop=(h == HPC - 1))
                        ob = obp.tile([P, 512], f32, tag="ob")
                        nc.scalar.activation(ob[:], pso3[:], AF.Copy)
                        nc.sync.dma_start(
                            out[tt * P:(tt + 1) * P, csl], ob[:])

    nc.finalize()
    return nc


def _get_compiled(seq, causal):
    key = (seq, causal)
    if key not in _compiled:
        _compiled[key] = _build(seq, causal)
    return _compiled[key]


def _rope_rmat():
    # lhsT for rot = Pmat @ q, Pmat[2i, 2i+1] = -1, Pmat[2i+1, 2i] = 1:
    # lhsT[d', d] = Pmat[d, d']
    m = np.zeros((D, D), dtype=np.float32)
    for i in range(D // 2):
        m[2 * i + 1, 2 * i] = -1.0
        m[2 * i, 2 * i + 1] = 1.0
    return m


def _prep_in_maps(x, inv_freqs, mask, Wq, Wk, Wv, Wo, seq):
    scale = 1.0 / math.sqrt(D)
    maskT = np.ascontiguousarray(mask.reshape(seq, seq).T)
    ifT = np.ascontiguousarray(inv_freqs.reshape(seq, D).T)
    rmat = _rope_rmat()

    shard = []
    for g in range(2):
        wqT = np.ascontiguousarray((Wq[g * 1024:(g + 1) * 1024, :] * scale).T)
        wkT = np.ascontiguousarray(Wk[g * 256:(g + 1) * 256, :].T)
        wvT = np.ascontiguousarray(Wv[g * 256:(g + 1) * 256, :].T)
        woT = np.ascontiguousarray(Wo[:, g * 1024:(g + 1) * 1024].T)
        shard.append((wqT, wkT, wvT, woT))

    in_maps = []
    for b in range(B):
        xTb = np.ascontiguousarray(x[b].T)
        for g in range(2):
            wqT, wkT, wvT, woT = shard[g]
            in_maps.append({
                "xT": xTb, "wqT": wqT, "wkT": wkT, "wvT": wvT, "woT": woT,
                "maskT": maskT, "ifT": ifT, "rmat": rmat,
            })
    return in_maps


def _check_causal(mask, seq):
    """True if blocks strictly above the diagonal may be skipped (mask very
    negative -> exp underflows to 0) and blocks strictly below the 128-block
    diagonal band need no mask add (mask exactly 0)."""
    m = mask.reshape(seq, seq)
    iu = np.triu_indices(seq, k=1)
    il = np.tril_indices(seq, k=0)
    return bool((m[iu] <= -1e4).all() and (m[il] == 0.0).all())


def kernel(x, start_pos, inv_freqs, mask, Wq, Wk, Wv, Wo, bo, _trace=False):
    from concourse.bass_utils import run_bass_kernel_spmd

    x = np.asarray(x, dtype=np.float32)
    inv_freqs = np.asarray(inv_freqs, dtype=np.float32)
    mask = np.asarray(mask, dtype=np.float32)
    Wq = np.asarray(Wq, dtype=np.float32)
    Wk = np.asarray(Wk, dtype=np.float32)
    Wv = np.asarray(Wv, dtype=np.float32)
    Wo = np.asarray(Wo, dtype=np.float32)
    bo = np.asarray(bo, dtype=np.float32)

    seq = x.shape[1]
    causal = _check_causal(mask, seq)
    nc = _get_compiled(seq, causal)
    in_maps = _prep_in_maps(x, inv_freqs, mask, Wq, Wk, Wv, Wo, seq)

    res = run_bass_kernel_spmd(nc, in_maps, core_ids=list(range(8)),
                               trace=_trace)
    outs = [r["out"] for r in res.results]
    y = np.empty((B, seq, C), dtype=np.float32)
    for b in range(B):
        y[b] = outs[2 * b] + outs[2 * b + 1] + bo[None, :]
    if _trace:
        kernel._last_results = res
    return y

